# revision 33
# baseline (speedup 1.0000x reference)
"""Trainium2 Bass kernel: grouped similarity-gating normalization.

Reference computation (per batch b, group g, cpg=64 channels, hw=784):
    means[c]  = mean_hw(x[c, :])
    s[hw]     = sum_c x[c, hw] * means[c]
    t         = (s - mean(s)) * rsqrt(var(s) + eps)
    gate      = sigmoid(t * weight[g] + bias[g])
    out[c,hw] = x[c, hw] * gate[hw]

Sharding: data-parallel over batch B=64 across 8 cores (8 batches/core).

Per-core layout: one SBUF tile [128, 4, 786] per batch holds channels
c = 4*p + j (p = partition, j = free chunk) -> contiguous 1.6MB DMAs, and
group(c) = c//64 = p//16, i.e. each group owns a 16-partition band.

Design (memory-regime; HBM floor ~54us/core: 12.8MB fp32 in + 6.4MB
fp16 out at ~358GB/s):
  - inputs cast fp32->bf16 DURING the DMA (SWDGE/gpsimd ring), all 8
    batches issued in the prologue -> input stream fully decoupled.
    bf16 halves SBUF traffic and unlocks 2x DVE packing + faster PE;
    the normalization is precision tolerant (harness gate 2e-2).
  - channel sums: j0 via DVE reduce, j1-3 via ACT Copy+accum_out
    written STRAIGHT into xt's padding column (copy lives in the
    sigmoid table set -> zero ACT table swaps in steady state).
  - s via PE: 4 accumulating bf16 matmuls, lhsT = indicator*sums
    (raw sums: t is scale-invariant, eps scaled by HW^2). The second
    chunk's sums column gives HW*mu for free.
  - variance via moments: ACT Square+accum (no bias -> fires the
    moment ps completes) + mu^2 on DVE; rstd = int32 magic-seed +
    1 Newton step, all on [NP,2] pair-wide tiles (2 batches share
    every small op).
  - gate = sigmoid(s*a + c), one ACT op, bf16 out.
  - gating multiply: ONE DVE tensor_tensor [128,4,784]
    bf16*bf16(broadcast)->fp16 in 2x packed mode; fp16 out halves
    write traffic; host upcasts.
  - two batches per pipeline slot (paired emission) amortize the
    cross-engine stats chain.
"""

import sys

if "/opt/trn_rl_repo" not in sys.path:
    sys.path.insert(0, "/opt/trn_rl_repo")

from contextlib import ExitStack

import numpy as np

import concourse.bacc as bacc
import concourse.tile as tile
from concourse import mybir
from concourse.bass_utils import run_bass_kernel_spmd

B, C, H, W = 64, 512, 28, 28
G = 8
HW = H * W          # 784
NCORES = 8
BLOC = B // NCORES  # 8 batches per core
NP = 128            # SBUF partitions
NJ = C // NP        # 4 channel chunks per partition (c = NJ*p + j)
PBAND = NP // G     # 16 partitions per group
EPS = 1e-5
F32 = mybir.dt.float32
BF16 = mybir.dt.bfloat16
F16 = mybir.dt.float16
I32 = mybir.dt.int32
MMCHUNK = 512       # max moving free dim per matmul (PSUM bank)
NR_ITERS = 1        # Newton iterations for rsqrt
MAGIC = 0x5F3759DF  # rsqrt seed constant
# lhsT uses raw channel sums (t is scale-invariant); eps scales by HW^2
EPS_EFF = float(HW) * float(HW) * EPS

N_DVE_SUMS = 2      # channel-sum j's via DVE reduce (rest: ACT copies)

_cache: dict = {}


def _emit(tc, nc, xs, m8, wv, bv, ys):
    AF = mybir.ActivationFunctionType
    OP = mybir.AluOpType
    with ExitStack() as ctx:
        consts = ctx.enter_context(tc.tile_pool(name="consts", bufs=1))
        xpool = ctx.enter_context(tc.tile_pool(name="xpool", bufs=BLOC))
        lpool = ctx.enter_context(tc.tile_pool(name="lpool", bufs=6))
        spool = ctx.enter_context(tc.tile_pool(name="spool", bufs=6))
        cpool = ctx.enter_context(tc.tile_pool(name="cpool", bufs=3))
        gpool = ctx.enter_context(tc.tile_pool(name="gpool", bufs=4))
        vpool = ctx.enter_context(tc.tile_pool(name="vpool", bufs=28))
        spsum = ctx.enter_context(tc.tile_pool(name="spsum", bufs=4, space="PSUM"))
        opool = ctx.enter_context(tc.tile_pool(name="opool", bufs=4))

        # M16[p, q] = (p//PBAND == q//PBAND) 0/1 indicator (exact in bf16);
        # wv/bv are the 16x-replicated per-partition weight/bias columns.
        m16_sb = consts.tile([NP, NP], BF16)
        nc.sync.dma_start(out=m16_sb[:], in_=m8[:])
        wv_sb = consts.tile([NP, 1], F32)
        nc.sync.dma_start(out=wv_sb[:], in_=wv[:])
        bv_sb = consts.tile([NP, 1], F32)
        nc.sync.dma_start(out=bv_sb[:], in_=bv[:])
        # dummy sigmoid so ACT's one table load is the sigmoid set (which
        # also holds copy/square) before real work arrives
        warm = consts.tile([NP, 1], F32)
        nc.vector.memset(warm[:], 0.0)
        nc.scalar.activation(out=warm[:], in_=warm[:], func=AF.Sigmoid)

        xts = {}
        state = {}

        def dma_in(b):
            # SWDGE ring casts fp32->bf16 in flight; two halves per batch.
            # 2 pad columns: col HW collects channel sums (accum_out) so the
            # matmul's second chunk emits HW*mu for free; col HW+1 is unread
            # filler keeping chunk widths even.
            xt = xpool.tile([NP, NJ, HW + 2], BF16)
            nc.gpsimd.dma_start(out=xt[:, 0:2, 0:HW], in_=xs[b, :, 0:2, :])
            nc.gpsimd.dma_start(out=xt[:, 2:4, 0:HW], in_=xs[b, :, 2:4, :])
            xts[b] = xt

        def phase1(b, n_dve_sums=N_DVE_SUMS):
            # channel sums (f32) + bf16 copy into xt's col HW + masked lhsT
            xt = xts[b]
            sums = spool.tile([NP, NJ], F32, tag="sums")
            if n_dve_sums:
                nc.vector.reduce_sum(
                    out=sums[:, 0:n_dve_sums],
                    in_=xt[:, 0:n_dve_sums, 0:HW],
                    axis=mybir.AxisListType.X,
                )
            cps = cpool.tile([NP, HW], BF16, tag="cps")
            for j in range(n_dve_sums, NJ):
                nc.scalar.activation(
                    out=cps[:], in_=xt[:, j, 0:HW], func=AF.Copy,
                    accum_out=sums[:, j : j + 1],
                )
            with nc.allow_low_precision(reason="bf16 mu column: gate 2e-2"):
                nc.vector.tensor_copy(
                    xt[:, :, HW : HW + 1], sums[:].unsqueeze(2)
                )
            lhsT = lpool.tile([NP, NJ, NP], BF16, tag="lhsT")
            for j in range(NJ):
                nc.vector.tensor_scalar_mul(
                    lhsT[:, j, :], m16_sb[:], sums[:, j : j + 1]
                )
            state[b] = lhsT

        def phase2(b):
            # s replicated onto each group's 16-partition band (M=128 free)
            xt = xts[b]
            lhsT = state[b]
            ps = spsum.tile([NP, HW + 2], F32)
            for c0 in range(0, HW + 2, MMCHUNK):
                c1 = min(c0 + MMCHUNK, HW + 2)
                for j in range(NJ):
                    nc.tensor.matmul(
                        ps[:, c0:c1], lhsT[:, j, :], xt[:, j, c0:c1],
                        start=(j == 0), stop=(j == NJ - 1),
                    )
            state[b] = ps

        def phase3_pair(b):
            # stats for the pair (b, b+1) on [NP,2] tiles: halves the DVE
            # small-op count. hwssq col k <- sum(s'^2) (ACT Square, no bias
            # -> fires as soon as ps is complete); mu ops run concurrently.
            ps_pair = (state[b], state[b + 1])
            sq = cpool.tile([NP, HW], F32, tag="sq")
            hws = vpool.tile([NP, 2], F32, tag="hws")
            nmu = vpool.tile([NP, 2], F32, tag="nmu")
            for k, ps in enumerate(ps_pair):
                nc.scalar.activation(
                    out=sq[:], in_=ps[:, 0:HW], func=AF.Square,
                    accum_out=hws[:, k : k + 1],
                )
                nc.vector.tensor_scalar_mul(
                    nmu[:, k : k + 1], ps[:, HW : HW + 1], -1.0 / HW
                )
            m2e = vpool.tile([NP, 2], F32, tag="m2e")
            nc.vector.tensor_mul(m2e[:], nmu[:], nmu[:])
            nc.vector.tensor_scalar_sub(m2e[:], m2e[:], EPS_EFF)
            u = vpool.tile([NP, 2], F32, tag="u")
            nc.vector.scalar_tensor_tensor(
                out=u[:], in0=hws[:], scalar=1.0 / HW, in1=m2e[:],
                op0=OP.mult, op1=OP.subtract,
            )
            # y0 = bits(MAGIC - (bits(u) >> 1)); K - t = (t ^ -1) + (K + 1)
            y = vpool.tile([NP, 2], F32, tag="y")
            yi = y[:].bitcast(I32)
            nc.vector.tensor_scalar(
                out=yi, in0=u[:].bitcast(I32), scalar1=1, scalar2=None,
                op0=OP.logical_shift_right,
            )
            nc.vector.tensor_scalar(
                out=yi, in0=yi, scalar1=-1, scalar2=None, op0=OP.bitwise_xor
            )
            nc.vector.tensor_scalar(
                out=yi, in0=yi, scalar1=MAGIC + 1, scalar2=None, op0=OP.add
            )
            h = vpool.tile([NP, 2], F32, tag="h")
            nc.vector.tensor_scalar_mul(h[:], u[:], -0.5)
            t = vpool.tile([NP, 2], F32, tag="t")
            for _ in range(NR_ITERS):
                nc.vector.tensor_mul(t[:], y[:], y[:])
                nc.vector.tensor_mul(t[:], t[:], h[:])
                nc.vector.scalar_tensor_tensor(
                    out=y[:], in0=t[:], scalar=1.5, in1=y[:],
                    op0=OP.add, op1=OP.mult,
                )
            a_t = vpool.tile([NP, 2], F32, tag="a_t")
            nc.vector.tensor_mul(a_t[:], y[:], wv_sb[:].to_broadcast([NP, 2]))
            c_t = vpool.tile([NP, 2], F32, tag="c_t")
            nc.vector.tensor_mul(c_t[:], nmu[:], a_t[:])
            nc.vector.tensor_add(c_t[:], c_t[:], bv_sb[:].to_broadcast([NP, 2]))
            for k, ps in enumerate(ps_pair):
                gate = gpool.tile([NP, HW], BF16, tag="gate")
                nc.scalar.activation(
                    out=gate[:], in_=ps[:, 0:HW], func=AF.Sigmoid,
                    bias=c_t[:, k : k + 1], scale=a_t[:, k : k + 1],
                )
                state[b + k] = gate

        def phase4(b, split=False):
            # gating multiply: bf16*bf16->fp16 2x-mode DVE; sync store.
            # split=True halves the op so the first store starts earlier
            # (used for the drain batches at the end).
            xt = xts.pop(b)
            gate = state.pop(b)
            ot = opool.tile([NP, NJ, HW], F16)
            gb = lambda n: gate[:].unsqueeze(1).to_broadcast([NP, n, HW])
            if split:
                nc.vector.tensor_mul(ot[:, 0:2, :], xt[:, 0:2, 0:HW], gb(2))
                nc.sync.dma_start(out=ys[b, :, 0:2, :], in_=ot[:, 0:2, :])
                nc.vector.tensor_mul(ot[:, 2:4, :], xt[:, 2:4, 0:HW], gb(2))
                nc.sync.dma_start(out=ys[b, :, 2:4, :], in_=ot[:, 2:4, :])
            else:
                nc.vector.tensor_mul(ot[:], xt[:, :, 0:HW], gb(NJ))
                nc.sync.dma_start(out=ys[b], in_=ot[:])

        # all inputs up front (xpool holds every batch; SWDGE ring drains
        # in order at HBM rate, decoupled from compute)
        for b in range(BLOC):
            dma_in(b)
        # fill: pair0 sums split DVE/ACT so the first matmul starts as
        # soon as batches 0-1 land (they head the SWDGE queue)
        phase1(0, n_dve_sums=2)
        phase1(1, n_dve_sums=2)
        phase2(0)
        phase2(1)
        # steady state: two batches per slot amortize the stats chain;
        # next pair's prep comes AFTER this pair's gate in every stream
        for b in range(0, BLOC, 2):
            phase3_pair(b)
            last = b + 2 >= BLOC
            phase4(b, split=last)
            phase4(b + 1, split=last)
            if not last:
                phase1(b + 2)
                phase1(b + 3)
                phase2(b + 2)
                phase2(b + 3)


def _build_nc():
    nc = bacc.Bacc("TRN2", debug=False)
    xs = nc.dram_tensor("xs", [BLOC, NP, NJ, HW], F32, kind="ExternalInput")
    m8 = nc.dram_tensor("m8", [NP, NP], BF16, kind="ExternalInput")
    wv = nc.dram_tensor("wv", [NP, 1], F32, kind="ExternalInput")
    bv = nc.dram_tensor("bv", [NP, 1], F32, kind="ExternalInput")
    ys = nc.dram_tensor("ys", [BLOC, NP, NJ, HW], F16, kind="ExternalOutput")
    with tile.TileContext(nc) as tc:
        _emit(tc, nc, xs, m8, wv, bv, ys)
    nc.compile()
    return nc


def get_nc():
    if "nc" not in _cache:
        _cache["nc"] = _build_nc()
    return _cache["nc"]


def make_in_maps(x, weight, bias):
    x = np.ascontiguousarray(np.asarray(x, dtype=np.float32))
    weight = np.asarray(weight, dtype=np.float32).reshape(G)
    bias = np.asarray(bias, dtype=np.float32).reshape(G)
    # [core, b, p, j, hw] with c = NJ*p + j
    xs = x.reshape(NCORES, BLOC, NP, NJ, HW)
    import ml_dtypes

    band = np.arange(NP) // PBAND
    m8 = (band[:, None] == band[None, :]).astype(ml_dtypes.bfloat16)
    wv = np.ascontiguousarray(np.repeat(weight, PBAND)[:, None])
    bv = np.ascontiguousarray(np.repeat(bias, PBAND)[:, None])
    return [
        {"xs": np.ascontiguousarray(xs[i]), "m8": m8, "wv": wv, "bv": bv}
        for i in range(NCORES)
    ]


def run(x, weight, bias, trace=False, **spmd_kwargs):
    nc = get_nc()
    in_maps = make_in_maps(x, weight, bias)
    res = run_bass_kernel_spmd(
        nc, in_maps, core_ids=list(range(NCORES)), trace=trace, **spmd_kwargs
    )
    out = np.stack(
        [res.results[i]["ys"].astype(np.float32) for i in range(NCORES)]
    )
    return out.reshape(B, C, H, W), res


def kernel(x, weight, bias, groups=G, **_ignored):
    assert int(groups) == G
    out, _ = run(x, weight, bias, trace=False)
    return out


# revision 34
# speedup vs baseline: 1.0661x; 1.0661x over previous
"""Trainium2 Bass kernel: grouped similarity-gating normalization.

Reference computation (per batch b, group g, cpg=64 channels, hw=784):
    means[c]  = mean_hw(x[c, :])
    s[hw]     = sum_c x[c, hw] * means[c]
    t         = (s - mean(s)) * rsqrt(var(s) + eps)
    gate      = sigmoid(t * weight[g] + bias[g])
    out[c,hw] = x[c, hw] * gate[hw]

Sharding: data-parallel over batch B=64 across 8 cores (8 batches/core).

Per-core layout: one SBUF tile [128, 4, 786] per batch holds channels
c = 4*p + j (p = partition, j = free chunk) -> contiguous 1.6MB DMAs, and
group(c) = c//64 = p//16, i.e. each group owns a 16-partition band.

Design (memory-regime; HBM floor ~54us/core: 12.8MB fp32 in + 6.4MB
fp16 out at ~358GB/s):
  - inputs cast fp32->bf16 DURING the DMA (SWDGE/gpsimd ring), all 8
    batches issued in the prologue -> input stream fully decoupled.
    bf16 halves SBUF traffic and unlocks 2x DVE packing + faster PE;
    the normalization is precision tolerant (harness gate 2e-2).
  - channel sums: j0 via DVE reduce, j1-3 via ACT Copy+accum_out
    written STRAIGHT into xt's padding column (copy lives in the
    sigmoid table set -> zero ACT table swaps in steady state).
  - s via PE: 4 accumulating bf16 matmuls, lhsT = indicator*sums
    (raw sums: t is scale-invariant, eps scaled by HW^2). The second
    chunk's sums column gives HW*mu for free.
  - variance via moments: ACT Square+accum (no bias -> fires the
    moment ps completes) + mu^2 on DVE; rstd = int32 magic-seed +
    1 Newton step, all on [NP,2] pair-wide tiles (2 batches share
    every small op).
  - gate = sigmoid(s*a + c), one ACT op, bf16 out.
  - gating multiply: ONE DVE tensor_tensor [128,4,784]
    bf16*bf16(broadcast)->fp16 in 2x packed mode; fp16 out halves
    write traffic; host upcasts.
  - two batches per pipeline slot (paired emission) amortize the
    cross-engine stats chain.
"""

import sys

if "/opt/trn_rl_repo" not in sys.path:
    sys.path.insert(0, "/opt/trn_rl_repo")

from contextlib import ExitStack

import numpy as np

import concourse.bacc as bacc
import concourse.tile as tile
from concourse import mybir
from concourse.bass_utils import run_bass_kernel_spmd

B, C, H, W = 64, 512, 28, 28
G = 8
HW = H * W          # 784
NCORES = 8
BLOC = B // NCORES  # 8 batches per core
NP = 128            # SBUF partitions
NJ = C // NP        # 4 channel chunks per partition (c = NJ*p + j)
PBAND = NP // G     # 16 partitions per group
EPS = 1e-5
F32 = mybir.dt.float32
BF16 = mybir.dt.bfloat16
F16 = mybir.dt.float16
I32 = mybir.dt.int32
MMCHUNK = 512       # max moving free dim per matmul (PSUM bank)
NR_ITERS = 1        # Newton iterations for rsqrt
MAGIC = 0x5F3759DF  # rsqrt seed constant
# lhsT uses raw channel sums (t is scale-invariant); eps scales by HW^2
EPS_EFF = float(HW) * float(HW) * EPS

N_DVE_SUMS = 2      # channel-sum j's via DVE reduce (rest: ACT copies)

_cache: dict = {}


def _emit(tc, nc, xs, m8, wv, bv, ys):
    AF = mybir.ActivationFunctionType
    OP = mybir.AluOpType
    with ExitStack() as ctx:
        consts = ctx.enter_context(tc.tile_pool(name="consts", bufs=1))
        xpool = ctx.enter_context(tc.tile_pool(name="xpool", bufs=BLOC))
        lpool = ctx.enter_context(tc.tile_pool(name="lpool", bufs=6))
        spool = ctx.enter_context(tc.tile_pool(name="spool", bufs=6))
        cpool = ctx.enter_context(tc.tile_pool(name="cpool", bufs=3))
        gpool = ctx.enter_context(tc.tile_pool(name="gpool", bufs=4))
        vpool = ctx.enter_context(tc.tile_pool(name="vpool", bufs=28))
        spsum = ctx.enter_context(tc.tile_pool(name="spsum", bufs=4, space="PSUM"))
        opool = ctx.enter_context(tc.tile_pool(name="opool", bufs=4))

        # M16[p, q] = (p//PBAND == q//PBAND) 0/1 indicator (exact in bf16);
        # wv/bv are the 16x-replicated per-partition weight/bias columns.
        m16_sb = consts.tile([NP, NP], BF16)
        nc.sync.dma_start(out=m16_sb[:], in_=m8[:])
        wv_sb = consts.tile([NP, 1], F32)
        nc.sync.dma_start(out=wv_sb[:], in_=wv[:])
        bv_sb = consts.tile([NP, 1], F32)
        nc.sync.dma_start(out=bv_sb[:], in_=bv[:])
        # dummy sigmoid so ACT's one table load is the sigmoid set (which
        # also holds copy/square) before real work arrives
        warm = consts.tile([NP, 1], F32)
        nc.vector.memset(warm[:], 0.0)
        nc.scalar.activation(out=warm[:], in_=warm[:], func=AF.Sigmoid)

        xts = {}
        state = {}

        def dma_in(b, quarters=False):
            # SWDGE ring casts fp32->bf16 in flight; two halves per batch
            # (quarters for the first pair -> the fill chain starts sooner).
            # 2 pad columns: col HW collects channel sums (accum_out) so the
            # matmul's second chunk emits HW*mu for free; col HW+1 is unread
            # filler keeping chunk widths even.
            xt = xpool.tile([NP, NJ, HW + 2], BF16)
            step = 1 if quarters else 2
            for j0 in range(0, NJ, step):
                nc.gpsimd.dma_start(
                    out=xt[:, j0 : j0 + step, 0:HW], in_=xs[b, :, j0 : j0 + step, :]
                )
            xts[b] = xt

        def phase1(b, n_dve_sums=N_DVE_SUMS):
            # channel sums (f32) + bf16 copy into xt's col HW + masked lhsT
            xt = xts[b]
            sums = spool.tile([NP, NJ], F32, tag="sums")
            if n_dve_sums:
                nc.vector.reduce_sum(
                    out=sums[:, 0:n_dve_sums],
                    in_=xt[:, 0:n_dve_sums, 0:HW],
                    axis=mybir.AxisListType.X,
                )
            cps = cpool.tile([NP, HW], BF16, tag="cps")
            for j in range(n_dve_sums, NJ):
                nc.scalar.activation(
                    out=cps[:], in_=xt[:, j, 0:HW], func=AF.Copy,
                    accum_out=sums[:, j : j + 1],
                )
            with nc.allow_low_precision(reason="bf16 mu column: gate 2e-2"):
                nc.vector.tensor_copy(
                    xt[:, :, HW : HW + 1], sums[:].unsqueeze(2)
                )
            lhsT = lpool.tile([NP, NJ, NP], BF16, tag="lhsT")
            for j in range(NJ):
                nc.vector.tensor_scalar_mul(
                    lhsT[:, j, :], m16_sb[:], sums[:, j : j + 1]
                )
            state[b] = lhsT

        def phase2(b):
            # s replicated onto each group's 16-partition band (M=128 free)
            xt = xts[b]
            lhsT = state[b]
            ps = spsum.tile([NP, HW + 2], F32)
            for c0 in (MMCHUNK, 0):
                c1 = min(c0 + MMCHUNK, HW + 2)
                for j in range(NJ):
                    nc.tensor.matmul(
                        ps[:, c0:c1], lhsT[:, j, :], xt[:, j, c0:c1],
                        start=(j == 0), stop=(j == NJ - 1),
                    )
            state[b] = ps

        def phase3_pair(b):
            # stats for the pair (b, b+1) on [NP,2] tiles: halves the DVE
            # small-op count. hwssq col k <- sum(s'^2) (ACT Square, no bias
            # -> fires as soon as ps is complete); mu ops run concurrently.
            ps_pair = (state[b], state[b + 1])
            sq = cpool.tile([NP, HW], F32, tag="sq")
            hws = vpool.tile([NP, 2], F32, tag="hws")
            nmu = vpool.tile([NP, 2], F32, tag="nmu")
            for k, ps in enumerate(ps_pair):
                nc.scalar.activation(
                    out=sq[:], in_=ps[:, 0:HW], func=AF.Square,
                    accum_out=hws[:, k : k + 1],
                )
                nc.vector.tensor_scalar_mul(
                    nmu[:, k : k + 1], ps[:, HW : HW + 1], -1.0 / HW
                )
            m2e = vpool.tile([NP, 2], F32, tag="m2e")
            nc.vector.tensor_mul(m2e[:], nmu[:], nmu[:])
            nc.vector.tensor_scalar_sub(m2e[:], m2e[:], EPS_EFF)
            u = vpool.tile([NP, 2], F32, tag="u")
            nc.vector.scalar_tensor_tensor(
                out=u[:], in0=hws[:], scalar=1.0 / HW, in1=m2e[:],
                op0=OP.mult, op1=OP.subtract,
            )
            # y0 = bits(MAGIC - (bits(u) >> 1)); K - t = (t ^ -1) + (K + 1)
            y = vpool.tile([NP, 2], F32, tag="y")
            yi = y[:].bitcast(I32)
            nc.vector.tensor_scalar(
                out=yi, in0=u[:].bitcast(I32), scalar1=1, scalar2=None,
                op0=OP.logical_shift_right,
            )
            nc.vector.tensor_scalar(
                out=yi, in0=yi, scalar1=-1, scalar2=None, op0=OP.bitwise_xor
            )
            nc.vector.tensor_scalar(
                out=yi, in0=yi, scalar1=MAGIC + 1, scalar2=None, op0=OP.add
            )
            h = vpool.tile([NP, 2], F32, tag="h")
            nc.vector.tensor_scalar_mul(h[:], u[:], -0.5)
            t = vpool.tile([NP, 2], F32, tag="t")
            for _ in range(NR_ITERS):
                nc.vector.tensor_mul(t[:], y[:], y[:])
                nc.vector.tensor_mul(t[:], t[:], h[:])
                nc.vector.scalar_tensor_tensor(
                    out=y[:], in0=t[:], scalar=1.5, in1=y[:],
                    op0=OP.add, op1=OP.mult,
                )
            a_t = vpool.tile([NP, 2], F32, tag="a_t")
            nc.vector.tensor_mul(a_t[:], y[:], wv_sb[:].to_broadcast([NP, 2]))
            c_t = vpool.tile([NP, 2], F32, tag="c_t")
            nc.vector.tensor_mul(c_t[:], nmu[:], a_t[:])
            nc.vector.tensor_add(c_t[:], c_t[:], bv_sb[:].to_broadcast([NP, 2]))
            for k, ps in enumerate(ps_pair):
                gate = gpool.tile([NP, HW], BF16, tag="gate")
                nc.scalar.activation(
                    out=gate[:], in_=ps[:, 0:HW], func=AF.Sigmoid,
                    bias=c_t[:, k : k + 1], scale=a_t[:, k : k + 1],
                )
                state[b + k] = gate

        def phase4(b, split=False):
            # gating multiply: bf16*bf16->fp16 2x-mode DVE; sync store.
            # split=True halves the op so the first store starts earlier
            # (used for the drain batches at the end).
            xt = xts.pop(b)
            gate = state.pop(b)
            ot = opool.tile([NP, NJ, HW], F16)
            gb = lambda n: gate[:].unsqueeze(1).to_broadcast([NP, n, HW])
            if split:
                nc.vector.tensor_mul(ot[:, 0:2, :], xt[:, 0:2, 0:HW], gb(2))
                nc.sync.dma_start(out=ys[b, :, 0:2, :], in_=ot[:, 0:2, :])
                nc.vector.tensor_mul(ot[:, 2:4, :], xt[:, 2:4, 0:HW], gb(2))
                nc.sync.dma_start(out=ys[b, :, 2:4, :], in_=ot[:, 2:4, :])
            else:
                nc.vector.tensor_mul(ot[:], xt[:, :, 0:HW], gb(NJ))
                nc.sync.dma_start(out=ys[b], in_=ot[:])

        # all inputs up front (xpool holds every batch; SWDGE ring drains
        # in order at HBM rate, decoupled from compute)
        for b in range(BLOC):
            dma_in(b, quarters=b < 2)
        # fill: pair0 sums split DVE/ACT so the first matmul starts as
        # soon as batches 0-1 land (they head the SWDGE queue)
        phase1(0, n_dve_sums=2)
        phase1(1, n_dve_sums=2)
        phase2(0)
        phase2(1)
        # steady state: two batches per slot amortize the stats chain;
        # next pair's prep comes AFTER this pair's gate in every stream
        for b in range(0, BLOC, 2):
            phase3_pair(b)
            last = b + 2 >= BLOC
            phase4(b, split=last)
            phase4(b + 1, split=last)
            if not last:
                phase1(b + 2)
                phase1(b + 3)
                phase2(b + 2)
                phase2(b + 3)


def _build_nc():
    nc = bacc.Bacc("TRN2", debug=False)
    xs = nc.dram_tensor("xs", [BLOC, NP, NJ, HW], F32, kind="ExternalInput")
    m8 = nc.dram_tensor("m8", [NP, NP], BF16, kind="ExternalInput")
    wv = nc.dram_tensor("wv", [NP, 1], F32, kind="ExternalInput")
    bv = nc.dram_tensor("bv", [NP, 1], F32, kind="ExternalInput")
    ys = nc.dram_tensor("ys", [BLOC, NP, NJ, HW], F16, kind="ExternalOutput")
    with tile.TileContext(nc) as tc:
        _emit(tc, nc, xs, m8, wv, bv, ys)
    nc.compile()
    return nc


def get_nc():
    if "nc" not in _cache:
        _cache["nc"] = _build_nc()
    return _cache["nc"]


def make_in_maps(x, weight, bias):
    x = np.ascontiguousarray(np.asarray(x, dtype=np.float32))
    weight = np.asarray(weight, dtype=np.float32).reshape(G)
    bias = np.asarray(bias, dtype=np.float32).reshape(G)
    # [core, b, p, j, hw] with c = NJ*p + j
    xs = x.reshape(NCORES, BLOC, NP, NJ, HW)
    import ml_dtypes

    band = np.arange(NP) // PBAND
    m8 = (band[:, None] == band[None, :]).astype(ml_dtypes.bfloat16)
    wv = np.ascontiguousarray(np.repeat(weight, PBAND)[:, None])
    bv = np.ascontiguousarray(np.repeat(bias, PBAND)[:, None])
    return [
        {"xs": np.ascontiguousarray(xs[i]), "m8": m8, "wv": wv, "bv": bv}
        for i in range(NCORES)
    ]


def run(x, weight, bias, trace=False, **spmd_kwargs):
    nc = get_nc()
    in_maps = make_in_maps(x, weight, bias)
    res = run_bass_kernel_spmd(
        nc, in_maps, core_ids=list(range(NCORES)), trace=trace, **spmd_kwargs
    )
    out = np.stack(
        [res.results[i]["ys"].astype(np.float32) for i in range(NCORES)]
    )
    return out.reshape(B, C, H, W), res


def kernel(x, weight, bias, groups=G, **_ignored):
    assert int(groups) == G
    out, _ = run(x, weight, bias, trace=False)
    return out
